# revision 1
# baseline (speedup 1.0000x reference)
"""Grok1-style MoE (T=2048, H=1024, E=8, I=2048, top-2) on 8 Trainium2 cores.

Strategy (expert-parallel, per the sharding hint):
  - Host: compute the tiny router (x @ gate_w, tanh softcap, top-2, softmax)
    and dispatch tokens by expert assignment (the "all-to-all dispatch" step:
    with full inputs on the host, dispatch = gather per expert), packing the
    per-core shards in the device-friendly tiled layout.
  - Device (SPMD, 1 expert per core): grouped GEMM
        gT = wg_e^T x_e^T ; uT = wu_e^T x_e^T   (computed transposed, [I, M])
        act = gelu_tanh(gT) * uT
        y_e = (act^T @ wd_e) * prob_e           ([M, H], row-scaled)
  - Host: combine = scatter-add per-expert outputs into [T, H].

All matmuls use float32r (fp32 data, FP22 multiply, fp32 accumulate) which
runs at ~1 cycle/row for moving dims >= 256.
"""

import numpy as np

import concourse.mybir as mybir
import concourse.tile as tile
from concourse import bacc
from concourse.bass_utils import run_bass_kernel_spmd

T, H, E, I_DIM, TOPK = 2048, 1024, 8, 2048, 2
SOFTCAP = 30.0
P = 128
N_CORES = 8
HC = 512        # h chunk (phase-2 matmul moving dim)
KH = H // P     # 8 contraction tiles (phase 1)
NI = I_DIM // P  # 16 i tiles
NH = H // HC    # 2 h chunks (phase 2)

_compiled = {}
LAST_RESULTS = None


def _m_chunks(M_PAD):
    """Split [0, M_PAD) into near-equal chunks <= 512 (>=256 when possible)."""
    n_chunks = max(1, -(-M_PAD // 512))
    base, rem = divmod(M_PAD, n_chunks)
    chunks, off = [], 0
    for c in range(n_chunks):
        ln = base + (1 if c < rem else 0)
        chunks.append((off, ln))
        off += ln
    return chunks


def _build(M_PAD):
    MT = M_PAD // P
    f32 = mybir.dt.float32
    f32r = mybir.dt.float32r

    nc = bacc.Bacc("TRN2", target_bir_lowering=False, num_devices=N_CORES)
    # Host-packed layouts (all DMAs contiguous per partition):
    #   xt  [KH, P, M_PAD]   : xt[k, p, m] = x_e[m, k*P+p]
    #   wg  [NI, P, KH*P]    : wg[it, p, k*P+i] = wg_e[k*P+p, it*P+i]
    #   wu  same as wg
    #   wd  [I, H]           : natural layout (row-tile slices are contiguous)
    xt = nc.dram_tensor("xt", [KH, P, M_PAD], f32r, kind="ExternalInput")
    wg = nc.dram_tensor("wg", [NI, P, KH * P], f32r, kind="ExternalInput")
    wu = nc.dram_tensor("wu", [NI, P, KH * P], f32r, kind="ExternalInput")
    wd = nc.dram_tensor("wd", [I_DIM, H], f32r, kind="ExternalInput")
    probs = nc.dram_tensor("probs", [P, MT], f32, kind="ExternalInput")
    y = nc.dram_tensor("y", [M_PAD, H], f32, kind="ExternalOutput")

    with tile.TileContext(nc) as tc:
        with (
            tc.tile_pool(name="persist", bufs=1) as persist,
            tc.tile_pool(name="wtiles", bufs=4) as wtiles,
            tc.tile_pool(name="outs", bufs=3) as outs,
            tc.tile_pool(name="psum", bufs=2, space="PSUM") as psum,
        ):
            xt_sb = persist.tile([P, KH, M_PAD], f32r)
            probs_sb = persist.tile([P, MT], f32)
            wd_sb = persist.tile([P, NI, H], f32r)
            acts = persist.tile([P, NI, M_PAD], f32r)

            def w_src(w, it):
                return w.ap()[it].rearrange("p (ko i) -> p ko i", i=P)

            # Startup feed: xt split across both HWDGE rings for bandwidth.
            half = KH // 2
            nc.sync.dma_start(
                xt_sb[:, :half], xt.ap()[:half].rearrange("k p m -> p k m")
            )
            nc.scalar.dma_start(
                xt_sb[:, half:], xt.ap()[half:].rearrange("k p m -> p k m")
            )
            nc.sync.dma_start(probs_sb[:], probs.ap())
            wg_sbs, wu_sbs = {}, {}
            wg_sbs[0] = wtiles.tile([P, KH, P], f32r, tag="wg", name="wg0")
            wu_sbs[0] = wtiles.tile([P, KH, P], f32r, tag="wu", name="wu0")
            for k0 in range(0, KH, 2):
                nc.sync.dma_start(wg_sbs[0][:, k0:k0 + 2], w_src(wg, 0)[:, k0:k0 + 2])
                nc.scalar.dma_start(wu_sbs[0][:, k0:k0 + 2], w_src(wu, 0)[:, k0:k0 + 2])

            # Phase 1: gT/uT = wg^T xT / wu^T xT per i-tile; act = gelu(g)*u.
            # wd tile loads are deferred to the back half of phase 1 (consumed
            # only in phase 2) to keep early bandwidth for wg/wu.
            for it in range(NI):
                if it not in wg_sbs:
                    wg_sbs[it] = wtiles.tile([P, KH, P], f32r, tag="wg", name=f"wg{it}")
                    nc.sync.dma_start(wg_sbs[it][:], w_src(wg, it))
                    wu_sbs[it] = wtiles.tile([P, KH, P], f32r, tag="wu", name=f"wu{it}")
                    nc.scalar.dma_start(wu_sbs[it][:], w_src(wu, it))
                wg_sb, wu_sb = wg_sbs.pop(it), wu_sbs.pop(it)
                if it >= NI - 8:
                    # two wd row-tiles per late iteration, alternating rings
                    for j in range(2):
                        wd_it = 2 * (it - (NI - 8)) + j
                        eng = nc.sync if j == 0 else nc.scalar
                        eng.dma_start(
                            wd_sb[:, wd_it], wd.ap()[wd_it * P:(wd_it + 1) * P, :]
                        )

                for (m0, ml) in _m_chunks(M_PAD):
                    g_ps = psum.tile([P, ml], f32, tag="g")
                    u_ps = psum.tile([P, ml], f32, tag="u")
                    for k in range(KH):
                        nc.tensor.matmul(
                            g_ps[:],
                            wg_sb[:, k],
                            xt_sb[:, k, m0:m0 + ml],
                            start=(k == 0),
                            stop=(k == KH - 1),
                        )
                    for k in range(KH):
                        nc.tensor.matmul(
                            u_ps[:],
                            wu_sb[:, k],
                            xt_sb[:, k, m0:m0 + ml],
                            start=(k == 0),
                            stop=(k == KH - 1),
                        )
                    nc.scalar.activation(
                        acts[:, it, m0:m0 + ml], g_ps[:],
                        mybir.ActivationFunctionType.Gelu_apprx_tanh,
                    )
                    nc.vector.tensor_mul(
                        acts[:, it, m0:m0 + ml], acts[:, it, m0:m0 + ml], u_ps[:]
                    )

            # Phase 2: y[m, h] = sum_i act[i, m] * wd[i, h], row-scaled by prob
            for mt in range(MT):
                for hc in range(NH):
                    d_ps = psum.tile([P, HC], f32, tag="d")
                    for it in range(NI):
                        nc.tensor.matmul(
                            d_ps[:],
                            acts[:, it, mt * P:(mt + 1) * P],
                            wd_sb[:, it, hc * HC:(hc + 1) * HC],
                            start=(it == 0),
                            stop=(it == NI - 1),
                        )
                    y_sb = outs.tile([P, HC], f32, tag="y")
                    nc.scalar.activation(
                        y_sb[:], d_ps[:],
                        mybir.ActivationFunctionType.Copy,
                        scale=probs_sb[:, mt:mt + 1],
                    )
                    nc.sync.dma_start(
                        y.ap()[mt * P:(mt + 1) * P, hc * HC:(hc + 1) * HC], y_sb[:]
                    )

    nc.compile()
    return nc


def _pack_w(w_e):
    """[H, I] -> [NI, P, KH*P] with w[it, p, k*P+i] = w_e[k*P+p, it*P+i]."""
    # [KH, P, NI, P_i] -> (NI, P, KH, P_i)
    w4 = w_e.reshape(KH, P, NI, P)
    return np.ascontiguousarray(w4.transpose(2, 1, 0, 3).reshape(NI, P, KH * P))


def kernel(hidden_states, gate_w, wg, wu, wd):
    global LAST_RESULTS
    x = np.ascontiguousarray(np.asarray(hidden_states, dtype=np.float32))
    gw = np.asarray(gate_w, dtype=np.float32)
    wg = np.asarray(wg, dtype=np.float32)
    wu = np.asarray(wu, dtype=np.float32)
    wd = np.asarray(wd, dtype=np.float32)

    # Router on host (part of the dispatch/sharding step).
    logits = np.tanh((x @ gw) / np.float32(SOFTCAP))
    top2 = np.argsort(-logits, axis=1, kind="stable")[:, :TOPK]  # [T, 2]
    v = np.take_along_axis(logits, top2, axis=1)                 # descending
    ex = np.exp(v - v[:, :1])
    pk = (ex / ex.sum(axis=1, keepdims=True)).astype(np.float32)  # [T, 2]

    token_ids, probs_e = [], []
    for e in range(E):
        mask = top2 == e
        rows = np.where(mask.any(axis=1))[0]
        kk = np.argmax(mask[rows], axis=1)
        token_ids.append(rows)
        probs_e.append(pk[rows, kk])

    n_max = max(len(r) for r in token_ids)
    M_PAD = max(P, -(-n_max // P) * P)
    MT = M_PAD // P

    nc = _compiled.get(M_PAD)
    if nc is None:
        nc = _build(M_PAD)
        _compiled[M_PAD] = nc

    in_maps = []
    for e in range(E):
        ids = token_ids[e]
        xe = np.zeros((M_PAD, H), np.float32)
        xe[: len(ids)] = x[ids]
        # [M_PAD, KH, P] -> [KH, P, M_PAD]
        xt_e = np.ascontiguousarray(xe.reshape(M_PAD, KH, P).transpose(1, 2, 0))
        pr = np.zeros((M_PAD,), np.float32)
        pr[: len(ids)] = probs_e[e]
        in_maps.append(
            {
                "xt": xt_e,
                "wg": _pack_w(wg[e]),
                "wu": _pack_w(wu[e]),
                "wd": np.ascontiguousarray(wd[e]),
                "probs": np.ascontiguousarray(pr.reshape(MT, P).T),
            }
        )

    res = run_bass_kernel_spmd(nc, in_maps, core_ids=list(range(N_CORES)))
    LAST_RESULTS = res

    out = np.zeros((T, H), np.float32)
    for e in range(E):
        ids = token_ids[e]
        out[ids] += res.results[e]["y"][: len(ids)]
    return out



# revision 3
# speedup vs baseline: 1.2009x; 1.2009x over previous
"""Grok1-style MoE (T=2048, H=1024, E=8, I=2048, top-2) on 8 Trainium2 cores.

Strategy (expert-parallel, per the sharding hint):
  - Host: compute the tiny router (x @ gate_w, tanh softcap, top-2, softmax)
    and dispatch tokens by expert assignment (the "all-to-all dispatch" step:
    with full inputs on the host, dispatch = gather per expert), packing the
    per-core shards in bf16 device-friendly layouts. The router prob of each
    (token, expert) pair is folded into the u-path copy of x, so the device
    never needs a separate prob-scaling pass:
        y_e^T = wd_e^T (gelu(wg_e^T x^T) * (wu_e^T (x*p)^T))
  - Device (SPMD, 1 expert per core), all matmuls bf16 (1 row/cycle, half
    the HBM traffic and LDWEIGHTS cost of fp32r):
      phase 1: gT = wg_e^T xg^T ; uT = wu_e^T xu^T   ([I, M], fp32 PSUM)
               act = gelu_tanh(gT) * uT              (bf16 SBUF)
      phase 2: yT[h, m] = sum_i wd_e[i, h] act[i, m] (wd stationary,
               act moving -> no M-padding to 128 needed)
  - Host: combine = scatter-add per-expert y^T into [T, H].
"""

import numpy as np
import ml_dtypes

import concourse.mybir as mybir
import concourse.tile as tile
from concourse import bacc
from concourse.bass_utils import run_bass_kernel_spmd

T, H, E, I_DIM, TOPK = 2048, 1024, 8, 2048, 2
SOFTCAP = 30.0
P = 128
N_CORES = 8
KH = H // P      # 8 contraction tiles (phase 1)
NI = I_DIM // P  # 16 i tiles
NH = H // P      # 8 h tiles (phase 2 output partition tiles)

BF16 = ml_dtypes.bfloat16

_compiled = {}
LAST_RESULTS = None


def _m_chunks(M_PAD):
    """Split [0, M_PAD) into near-equal chunks <= 512 (PSUM bank: 512 fp32)."""
    n_chunks = max(1, -(-M_PAD // 512))
    base = -(-(M_PAD // n_chunks) // 8) * 8
    chunks, off = [], 0
    for _ in range(n_chunks):
        ln = min(base, M_PAD - off)
        chunks.append((off, ln))
        off += ln
    return [c for c in chunks if c[1] > 0]


def _build(M_PAD):
    f32 = mybir.dt.float32
    bf16 = mybir.dt.bfloat16
    chunks = _m_chunks(M_PAD)

    nc = bacc.Bacc("TRN2", target_bir_lowering=False, num_devices=N_CORES)
    # Host-packed layouts (all DMAs contiguous per partition):
    #   xg  [P, KH, M_PAD] : xg[p, k, m] = x_e[m, k*P+p]            (bf16)
    #   xu  same, but x pre-scaled by the router prob of each token (bf16)
    #   wg  [NI, P, KH*P]  : wg[it, p, k*P+i] = wg_e[k*P+p, it*P+i] (bf16)
    #   wu  same as wg
    #   wd  [I, H]         : natural layout                          (bf16)
    #   y   [NH, P, M_PAD] : y[h, p, m] = out_e[m, h*P+p]            (fp32)
    xg = nc.dram_tensor("xg", [P, KH, M_PAD], bf16, kind="ExternalInput")
    xu = nc.dram_tensor("xu", [P, KH, M_PAD], bf16, kind="ExternalInput")
    wg = nc.dram_tensor("wg", [NI, P, KH * P], bf16, kind="ExternalInput")
    wu = nc.dram_tensor("wu", [NI, P, KH * P], bf16, kind="ExternalInput")
    wd = nc.dram_tensor("wd", [I_DIM, H], bf16, kind="ExternalInput")
    y = nc.dram_tensor("y", [NH, P, M_PAD], f32, kind="ExternalOutput")

    with tile.TileContext(nc) as tc:
        with (
            tc.tile_pool(name="persist", bufs=1) as persist,
            tc.tile_pool(name="wtiles", bufs=3) as wtiles,
            tc.tile_pool(name="gelu", bufs=2) as gpool,
            tc.tile_pool(name="outs", bufs=3) as outs,
            tc.tile_pool(name="psum", bufs=2, space="PSUM") as psum,
        ):
            xg_sb = persist.tile([P, KH, M_PAD], bf16)
            xu_sb = persist.tile([P, KH, M_PAD], bf16)
            wd_sb = persist.tile([P, NI, H], bf16)
            acts = persist.tile([P, NI, M_PAD], bf16)

            def w_src(w, it):
                return w.ap()[it].rearrange("p (ko i) -> p ko i", i=P)

            wg_sbs, wu_sbs = {}, {}

            def load_w(it):
                wg_sbs[it] = wtiles.tile([P, KH, P], bf16, tag="wg", name=f"wg{it}")
                wu_sbs[it] = wtiles.tile([P, KH, P], bf16, tag="wu", name=f"wu{it}")
                half = KH // 2
                nc.gpsimd.dma_start(wg_sbs[it][:, :half], w_src(wg, it)[:, :half])
                nc.gpsimd.dma_start(wg_sbs[it][:, half:], w_src(wg, it)[:, half:])
                nc.scalar.dma_start(wu_sbs[it][:, :half], w_src(wu, it)[:, :half])
                nc.scalar.dma_start(wu_sbs[it][:, half:], w_src(wu, it)[:, half:])

            # Startup: xg in per-k-pair pieces (first matmul fires after the
            # first piece + first wg half), then xu; weights stream on the
            # gpsimd/scalar rings in consumption order.
            load_w(0)
            for kk in range(0, KH, 2):
                nc.sync.dma_start(xg_sb[:, kk:kk + 2], xg.ap()[:, kk:kk + 2])
            load_w(1)
            for kk in range(0, KH, 2):
                nc.sync.dma_start(xu_sb[:, kk:kk + 2], xu.ap()[:, kk:kk + 2])

            # Phase 1: gT/uT = wg^T xg / wu^T xu per i-tile; act = gelu(g)*u.
            # wd tile loads interleave on the sync ring mid-phase (consumed
            # only in phase 2), keeping early bandwidth for xg/xu/wg/wu.
            for it in range(NI):
                if it + 2 < NI:
                    load_w(it + 2)
                if 4 <= it < 12:
                    for j in range(2):
                        wd_it = 2 * (it - 4) + j
                        nc.sync.dma_start(
                            wd_sb[:, wd_it], wd.ap()[wd_it * P:(wd_it + 1) * P, :]
                        )
                wg_sb, wu_sb = wg_sbs.pop(it), wu_sbs.pop(it)

                for (m0, ml) in chunks:
                    g_ps = psum.tile([P, ml], f32, tag="g")
                    for k in range(KH):
                        nc.tensor.matmul(
                            g_ps[:],
                            wg_sb[:, k],
                            xg_sb[:, k, m0:m0 + ml],
                            start=(k == 0),
                            stop=(k == KH - 1),
                        )
                    u_ps = psum.tile([P, ml], f32, tag="u")
                    for k in range(KH):
                        nc.tensor.matmul(
                            u_ps[:],
                            wu_sb[:, k],
                            xu_sb[:, k, m0:m0 + ml],
                            start=(k == 0),
                            stop=(k == KH - 1),
                        )
                    t = gpool.tile([P, ml], bf16, tag="t")
                    nc.scalar.activation(
                        t[:], g_ps[:], mybir.ActivationFunctionType.Gelu_apprx_tanh
                    )
                    nc.vector.tensor_mul(acts[:, it, m0:m0 + ml], t[:], u_ps[:])

            # Phase 2: yT[h, m] = sum_i wd[i, h] * act[i, m]
            # (wd tile stationary, act moving; probs already folded into u).
            for h in range(NH):
                for ci, (m0, ml) in enumerate(chunks):
                    y_ps = psum.tile([P, ml], f32, tag="y")
                    for it in range(NI):
                        nc.tensor.matmul(
                            y_ps[:],
                            wd_sb[:, it, h * P:(h + 1) * P],
                            acts[:, it, m0:m0 + ml],
                            start=(it == 0),
                            stop=(it == NI - 1),
                        )
                    y_sb = outs.tile([P, ml], f32, tag="y")
                    nc.scalar.activation(
                        y_sb[:], y_ps[:], mybir.ActivationFunctionType.Copy
                    )
                    eng = nc.sync if (h * len(chunks) + ci) % 2 == 0 else nc.gpsimd
                    eng.dma_start(y.ap()[h, :, m0:m0 + ml], y_sb[:])

    nc.compile()
    return nc


def _pack_w(w_e):
    """[H, I] -> [NI, P, KH*P] bf16 with w[it, p, k*P+i] = w_e[k*P+p, it*P+i]."""
    w4 = w_e.reshape(KH, P, NI, P)
    return np.ascontiguousarray(
        w4.transpose(2, 1, 0, 3).reshape(NI, P, KH * P).astype(BF16)
    )


def kernel(hidden_states, gate_w, wg, wu, wd):
    global LAST_RESULTS
    x = np.ascontiguousarray(np.asarray(hidden_states, dtype=np.float32))
    gw = np.asarray(gate_w, dtype=np.float32)
    wg = np.asarray(wg, dtype=np.float32)
    wu = np.asarray(wu, dtype=np.float32)
    wd = np.asarray(wd, dtype=np.float32)

    # Router on host (part of the dispatch/sharding step).
    logits = np.tanh((x @ gw) / np.float32(SOFTCAP))
    top2 = np.argsort(-logits, axis=1, kind="stable")[:, :TOPK]  # [T, 2]
    v = np.take_along_axis(logits, top2, axis=1)                 # descending
    ex = np.exp(v - v[:, :1])
    pk = (ex / ex.sum(axis=1, keepdims=True)).astype(np.float32)  # [T, 2]

    token_ids, probs_e = [], []
    for e in range(E):
        mask = top2 == e
        rows = np.where(mask.any(axis=1))[0]
        kk = np.argmax(mask[rows], axis=1)
        token_ids.append(rows)
        probs_e.append(pk[rows, kk])

    n_max = max(len(r) for r in token_ids)
    M_PAD = max(64, -(-n_max // 16) * 16)

    nc = _compiled.get(M_PAD)
    if nc is None:
        nc = _build(M_PAD)
        _compiled[M_PAD] = nc

    in_maps = []
    for e in range(E):
        ids = token_ids[e]
        xe = np.zeros((M_PAD, H), np.float32)
        xe[: len(ids)] = x[ids]
        xue = np.zeros((M_PAD, H), np.float32)
        xue[: len(ids)] = x[ids] * probs_e[e][:, None]
        # [M_PAD, KH, P] -> [P, KH, M_PAD]
        xg_e = np.ascontiguousarray(
            xe.reshape(M_PAD, KH, P).transpose(2, 1, 0).astype(BF16)
        )
        xu_e = np.ascontiguousarray(
            xue.reshape(M_PAD, KH, P).transpose(2, 1, 0).astype(BF16)
        )
        in_maps.append(
            {
                "xg": xg_e,
                "xu": xu_e,
                "wg": _pack_w(wg[e]),
                "wu": _pack_w(wu[e]),
                "wd": np.ascontiguousarray(wd[e].astype(BF16)),
            }
        )

    res = run_bass_kernel_spmd(nc, in_maps, core_ids=list(range(N_CORES)))
    LAST_RESULTS = res

    out = np.zeros((T, H), np.float32)
    for e in range(E):
        ids = token_ids[e]
        yT = res.results[e]["y"].reshape(H, M_PAD)          # [H, M_PAD]
        out[ids] += yT[:, : len(ids)].T
    return out
